# revision 1
# baseline (speedup 1.0000x reference)
"""Trainium2 Bass kernel for nn_MultiHeadAttention_76587856823057.

Sharding: (batch, query-half) -> 8 cores, zero collectives.
Per core: b fixed, queries TQ=1024 (half of T), all H=16 heads, all TK=2048 keys.

v2 design notes:
 - softmax is shift-invariant; the reference's *global* max subtract cancels in
   the normalization; scores are bounded so exp() cannot overflow in fp32.
 - exp(s*m)*m == exp(s)*m for m in {0,1}: one mask multiply only (after exp).
 - QK contracts K=64 per head using PE row-tiling (tile_position): head-even on
   array rows 0-63, head-odd on rows 64-127, running CONCURRENTLY -> ~2x QK.
 - row sums come free from the PV matmul via a ones-column per head (M=65);
   normalization (1/rowsum) is fused into the PV evacuation: DVE
   reciprocal_approx_fast (via an SBUF row copy - the custom op mishandles
   partition-offset PSUM reads) + gpsimd partition_broadcast (gpsimd runs
   nothing else, so its library loads once).  Evacuation is software-pipelined
   into the next (pair, q-half) pass (emitted at kt==2) so it never stalls the
   PE FIFO.
 - attention output stays in SBUF (bf16) and feeds the O-projection as the
   STATIONARY operand, producing x in [q, d] layout directly -> LayerNorm needs
   no transposes at all.
 - all weights/activations bf16 (residual path fp32); biases folded in via
   ones-row K=1 matmuls.
 - K-projection and V-projection are interleaved into the ACT(exp)-bound
   attention phase to fill PE slack; all weight pools are pre-opened so their
   DMAs never wait on earlier pools' readers.
 - PSUM: qk pool 3x[128,1024] (6 banks, also serves Q/K/V-proj psum) +
   pv0/pv1 [65,512] (2 banks) = 8 banks.  Deep qk rotation keeps the PE FIFO
   fed (HAM clock gate needs sustained activity to stay at 2.4 GHz).
 - startup: ~72 dependency-free warm-up matmuls run during the initial weight
   DMA so the PE is at full clock when Q-proj issues; DMAs split across the
   sync and scalar HWDGE queues; Wo loads mid-attention into SBUF freed by
   the V-proj pools (closed after pass 0).

Self-contained: hardcodes all shapes; no sibling imports.
"""

import os
import numpy as np

import concourse.bass as bass
from concourse import bacc
import concourse.mybir as mybir
from concourse.tile import TileContext
from concourse.bass_utils import run_bass_kernel_spmd

F32 = mybir.dt.float32
BF16 = mybir.dt.bfloat16
AF = mybir.ActivationFunctionType

B, T, D, H, DK = 4, 2048, 1024, 16, 64
TQ = T // 2          # queries per core
TK = T               # keys per core
NCORES = 8
NPAIR = H // 2       # 8 head pairs
NFT = D // 128       # 8 feature tiles
NKT = TK // 128      # 16 key tiles
VEXT = H * (DK + 1)  # 1040: per-head [64 v-cols + ones col]

_LAST_RESULTS = {}


def build_program(nc: bass.Bass, trivial_affine: bool = False,
                  trivial_bias: bool = False):
    # ---- per-core DRAM I/O ----
    qT = nc.dram_tensor("qT", [D + 1, TQ], BF16, kind="ExternalInput").ap()
    kT = nc.dram_tensor("kT", [D + 1, TK], BF16, kind="ExternalInput").ap()
    vT = nc.dram_tensor("vT", [D + 1, TK], BF16, kind="ExternalInput").ap()
    wq = nc.dram_tensor("wq", [D + 1, D], BF16, kind="ExternalInput").ap()
    wk = nc.dram_tensor("wk", [D + 1, D], BF16, kind="ExternalInput").ap()
    wv = nc.dram_tensor("wv", [D + 1, VEXT], BF16, kind="ExternalInput").ap()
    wo = nc.dram_tensor("wo", [D + 1, D], BF16, kind="ExternalInput").ap()
    maskT = nc.dram_tensor("maskT", [TK, TQ], BF16, kind="ExternalInput").ap()
    qres = nc.dram_tensor("qres", [TQ, D], F32, kind="ExternalInput").ap()
    gam = nc.dram_tensor("gam", [1, D], F32, kind="ExternalInput").ap()
    bet = nc.dram_tensor("bet", [1, D], F32, kind="ExternalInput").ap()
    out = nc.dram_tensor("out", [TQ, D], F32, kind="ExternalOutput").ap()

    with TileContext(nc) as tc:
        import contextlib
        with contextlib.ExitStack() as ctx:
            pers = ctx.enter_context(tc.tile_pool(name="pers", bufs=1))

            qhT = pers.tile([128, NFT, TQ], BF16)        # 16 KB/part
            vh_sb = pers.tile([128, NKT, VEXT], BF16)    # 32.5 KB/part
            mk = pers.tile([128, NKT, TQ], BF16)         # 32 KB/part
            attn_sb = pers.tile([128, NPAIR, TQ], BF16)  # 16 KB/part
            ones = pers.tile([1, TK], BF16)              # ones (bias mms)

            # PSUM pools (8 banks): qk 2x2 + (pv0,pv1) 1x1 each + aux 2x1
            apsum = ctx.enter_context(contextlib.ExitStack())
            qkps = apsum.enter_context(
                tc.tile_pool(name="qkps", bufs=3, space="PSUM"))
            pvps = apsum.enter_context(
                tc.tile_pool(name="pvps", bufs=1, space="PSUM"))

            # Long-lived attention-phase pools opened up front: their DMAs
            # have no SBUF-reuse dependency and start as soon as the queue
            # reaches them.  Emission order == transfer order on the queue.
            asbuf = ctx.enter_context(contextlib.ExitStack())
            pepool = asbuf.enter_context(tc.tile_pool(name="pepool", bufs=3))
            pmpool = asbuf.enter_context(tc.tile_pool(name="pmpool", bufs=8 if trivial_bias else 6))
            evpool = asbuf.enter_context(tc.tile_pool(name="evpool", bufs=1))
            kw = asbuf.enter_context(tc.tile_pool(name="kwpool", bufs=1))
            kqpool = asbuf.enter_context(tc.tile_pool(name="kqpool", bufs=4))
            khpool = asbuf.enter_context(tc.tile_pool(name="khpool", bufs=3))
            vstack = contextlib.ExitStack()
            vw = vstack.enter_context(tc.tile_pool(name="vwpool", bufs=1))
            vstage = vstack.enter_context(tc.tile_pool(name="vstage", bufs=12))

            nc.vector.memset(ones, 1.0)

            # HAM warm-up: dependency-free matmuls fill the PE during the
            # initial weight DMA so the clock gate is at 8/8 (2.4 GHz) when
            # the first projection matmul issues (~3.4us of sustained PE
            # activity releases the throttle; idle would hold it at 1.2 GHz).
            wtile = qkps.tile([128, 1024], F32, tag="qk", name="warm")
            for _ in range(44):
                nc.tensor.matmul(wtile[0:64, 0:512], ones[0:1, 0:64],
                                 ones[0:1, 0:512], start=True, stop=True)

            # ---------------- Q projection -> qhT resident ------------------
            with tc.tile_pool(name="qppool", bufs=1) as qp:
                wq_m = qp.tile([128, NFT, D], BF16, tag="wq_m")
                qT_m = qp.tile([128, NFT, TQ], BF16, tag="qT_m")
                nc.sync.dma_start(
                    out=wq_m, in_=wq[0:D, :].rearrange("(k p) f -> p k f", p=128))
                nc.scalar.dma_start(
                    out=qT_m, in_=qT[0:D, :].rearrange("(k p) t -> p k t", p=128))
                if not trivial_bias:
                    wq_b = qp.tile([1, D], BF16, tag="wq_b")
                    nc.sync.dma_start(out=wq_b, in_=wq[D:D + 1, :])

                # weight DMAs for K/V/mask issued next on the queue
                wk_m = kw.tile([128, NFT, D], BF16, tag="wk_m")
                nc.sync.dma_start(
                    out=wk_m, in_=wk[0:D, :].rearrange("(k p) f -> p k f", p=128))
                if not trivial_bias:
                    wk_b = kw.tile([1, D], BF16, tag="wk_b")
                    nc.sync.dma_start(out=wk_b, in_=wk[D:D + 1, :])
                wv_m = vw.tile([128, NFT, VEXT], BF16, tag="wv_m")
                wv_b = vw.tile([1, VEXT], BF16, tag="wv_b")
                nc.sync.dma_start(
                    out=wv_m, in_=wv[0:D, :].rearrange("(k p) f -> p k f", p=128))
                nc.sync.dma_start(out=wv_b, in_=wv[D:D + 1, :])
                nc.scalar.dma_start(
                    out=mk, in_=maskT.rearrange("(t p) q -> p t q", p=128))

                for c in range(2):
                    cs = slice(c * 512, (c + 1) * 512)
                    for fi in range(NFT):
                        fs = slice(fi * 128, (fi + 1) * 128)
                        ps_t = qkps.tile([128, 1024], F32, tag="qk", name="qk")
                        ps = ps_t[:, 0:512]
                        for ki in range(NFT):
                            nc.tensor.matmul(ps, wq_m[:, ki, fs], qT_m[:, ki, cs],
                                             start=(ki == 0),
                                             stop=(trivial_bias and ki == NFT - 1))
                        if not trivial_bias:
                            nc.tensor.matmul(ps, wq_b[0:1, fs], ones[0:1, cs],
                                             start=False, stop=True)
                        if fi % 2 == 0:
                            nc.scalar.copy(qhT[:, fi, cs], ps)
                        else:
                            nc.vector.tensor_copy(qhT[:, fi, cs], ps)

            # ---------------- K / V projection emitters ---------------------
            def emit_kproj_qtr(j, khp, qtr):
                # khp[:, qtr] = (k @ Wk + bk).T rows j*128.., key qtr slice
                qs = slice(qtr * 512, (qtr + 1) * 512)
                ps_t = qkps.tile([128, 1024], F32, tag="qk", name="qk")
                ps = ps_t[:, 0:512]
                fs = slice(j * 128, (j + 1) * 128)
                for ki in range(NFT):
                    kT_q = kqpool.tile([128, 512], BF16, tag="ktq", name="ktq")
                    nc.sync.dma_start(
                        out=kT_q, in_=kT[ki * 128:(ki + 1) * 128, qs])
                    nc.tensor.matmul(ps, wk_m[:, ki, fs], kT_q,
                                     start=(ki == 0),
                                     stop=(trivial_bias and ki == NFT - 1))
                if not trivial_bias:
                    nc.tensor.matmul(ps, wk_b[0:1, fs], ones[0:1, qs],
                                     start=False, stop=True)
                nc.vector.tensor_copy(khp[:, qs], ps)

            VCH = [(0, 512), (512, 1024), (1024, VEXT)]

            def emit_vproj_ti(ti):
                # vh_sb[:, ti, :] = (v @ Wv_ext + bv_ext).T tile ti (128 keys)
                tsl = slice(ti * 128, (ti + 1) * 128)
                vT_k = []
                for ki in range(NFT):
                    vt = vstage.tile([128, 128], BF16, tag="vT_m", name="vTm")
                    nc.scalar.dma_start(
                        out=vt, in_=vT[ki * 128:(ki + 1) * 128, tsl])
                    vT_k.append(vt)
                for (c0, c1) in VCH:
                    ps_t = qkps.tile([128, 1024], F32, tag="qk", name="qk")
                    ps = ps_t[:, 0:512]
                    n = c1 - c0
                    for ki in range(NFT):
                        nc.tensor.matmul(ps[:, 0:n], vT_k[ki],
                                         wv_m[:, ki, c0:c1],
                                         start=(ki == 0), stop=False)
                    nc.tensor.matmul(ps[:, 0:n], ones[0:1, tsl],
                                     wv_b[0:1, c0:c1], start=False, stop=True)
                    if c0 == 0:
                        nc.scalar.copy(vh_sb[:, ti, c0:c1], ps[:, 0:n])
                    else:
                        nc.vector.tensor_copy(vh_sb[:, ti, c0:c1], ps[:, 0:n])

            # ---------------- attention pass pipeline -----------------------
            khps = [None] * NPAIR
            khps[0] = khpool.tile([128, TK], BF16, tag="khp", name="khp")
            for qtr in range(4):
                emit_kproj_qtr(0, khps[0], qtr)
            emit_vproj_ti(0)
            emit_vproj_ti(1)

            pend_evac = [None]

            def run_pass(pi):
                j, qh = pi // 2, pi % 2
                khp = khps[j]
                qsl = slice(qh * 512, (qh + 1) * 512)
                h0sl = slice((2 * j) * 65, (2 * j) * 65 + 65)
                h1sl = slice((2 * j + 1) * 65, (2 * j + 1) * 65 + 65)
                lag = 2 if pi == 0 else 3
                pv0 = pvps.tile([65, 512], F32, tag="pv0", name="pv0")
                pv1 = pvps.tile([65, 512], F32, tag="pv1", name="pv1")
                work = []

                def emit_pv():
                    pm0, pm1, kt = work.pop(0)
                    nc.tensor.matmul(pv0, vh_sb[:, kt, h0sl], pm0,
                                     start=(kt == 0), stop=(kt == NKT - 1))
                    nc.tensor.matmul(pv1, vh_sb[:, kt, h1sl], pm1,
                                     start=(kt == 0), stop=(kt == NKT - 1))

                qks = {}

                def emit_qk(kt):
                    tsl = slice(kt * 128, (kt + 1) * 128)
                    qk = qkps.tile([128, 1024], F32, tag="qk", name="qk")
                    # concurrent row-tiled QK: h-even rows 0-63, h-odd 64-127
                    nc.tensor.matmul(qk[:, 0:512], khp[0:64, tsl],
                                     qhT[0:64, j, qsl], start=True, stop=True)
                    nc.tensor.matmul(qk[:, 512:1024], khp[64:128, tsl],
                                     qhT[64:128, j, qsl], start=True, stop=True)
                    qks[kt] = qk

                emit_qk(0)
                for kt in range(NKT):
                    # QK one iteration ahead: keeps the exp stream in front
                    # of interleaved kproj/vproj/PV work in the PE FIFO
                    if kt + 1 < NKT:
                        emit_qk(kt + 1)
                    qk = qks.pop(kt)
                    pe = pepool.tile([128, 1024], BF16, tag="pe", name="pe")
                    nc.scalar.activation(pe, qk, AF.Exp)
                    pm0 = pmpool.tile([128, 512], BF16, tag="pm", name="pm")
                    pm1 = pmpool.tile([128, 512], BF16, tag="pm", name="pm")
                    nc.vector.tensor_mul(pm0, pe[:, 0:512], mk[:, kt, qsl])
                    nc.vector.tensor_mul(pm1, pe[:, 512:1024], mk[:, kt, qsl])
                    work.append((pm0, pm1, kt))
                    # software-pipelined evac of the previous pass
                    if kt == 2 and pend_evac[0] is not None:
                        pend_evac[0]()
                        pend_evac[0] = None
                    # interleaved projection work (fills PE slack)
                    if pi == 0 and kt < 14:
                        emit_vproj_ti(kt + 2)
                    # K-proj of the next pair, spread across both passes of
                    # pair j (pair 1 stays in pass 1: pass 0 is vproj-bound)
                    kq = None
                    if pi == 1 and kt in (3, 7, 11, 14):
                        kq = (3, 7, 11, 14).index(kt)
                    elif pi >= 2 and j < NPAIR - 1 and kt in (5, 11):
                        kq = (0 if kt == 5 else 1) + 2 * (pi % 2)
                    if kq is not None and j < NPAIR - 1:
                        if khps[j + 1] is None:
                            khps[j + 1] = khpool.tile([128, TK], BF16,
                                                      tag="khp", name="khp")
                        emit_kproj_qtr(j + 1, khps[j + 1], kq)
                    tgt = min(lag, NKT - 1 - kt)
                    while len(work) > tgt:
                        emit_pv()
                while work:
                    emit_pv()

                def evac():
                    for hh, pv in ((0, pv0), (1, pv1)):
                        rs = evpool.tile([1, 512], F32, tag="rs", name="rs")
                        nc.vector.tensor_copy(rs[0:1, :], pv[64:65, :])
                        rr = evpool.tile([1, 512], F32, tag="rr", name="rr")
                        nc.vector.reciprocal_approx_fast(rr[0:1, :],
                                                         rs[0:1, :])
                        rrb = evpool.tile([64, 512], F32, tag="rrb", name="rrb")
                        nc.gpsimd.partition_broadcast(rrb, rr[0:1, :])
                        nc.vector.tensor_mul(
                            attn_sb[64 * hh:64 * hh + 64, j, qsl],
                            pv[0:64, :], rrb)
                pend_evac[0] = evac

            cmref = [None]
            for pi in range(2 * NPAIR):
                run_pass(pi)
                if pi == 0:
                    vstack.close()
                    cm = ctx.enter_context(tc.tile_pool(name="cm", bufs=1))
                    cmref[0] = cm
                    wo_m = cm.tile([128, NFT, D], BF16)
                    nc.scalar.dma_start(
                        out=wo_m[:, :, 0:512],
                        in_=wo[0:D, 0:512].rearrange("(k p) f -> p k f", p=128))
                    nc.sync.dma_start(
                        out=wo_m[:, :, 512:1024],
                        in_=wo[0:D, 512:1024].rearrange("(k p) f -> p k f", p=128))
                    if not trivial_bias:
                        wo_b = cm.tile([1, D], BF16)
                        nc.sync.dma_start(out=wo_b, in_=wo[D:D + 1, :])
            pend_evac[0]()
            pend_evac[0] = None
            cm = cmref[0]

            # free attention-phase SBUF + PSUM before phase C
            apsum.close()

            # ------------ phase C: out-proj + residual + LN -----------------
            with tc.tile_pool(name="cq", bufs=2) as cq, \
                 tc.tile_pool(name="cl", bufs=2) as cl, \
                 tc.tile_pool(name="cps", bufs=2, space="PSUM") as cps:

                eps_t = cm.tile([128, 1], F32)
                nc.vector.memset(eps_t, 1e-5)
                if not trivial_affine:
                    gam_r = cm.tile([1, D], F32)
                    bet_r = cm.tile([1, D], F32)
                    nc.sync.dma_start(out=gam_r, in_=gam)
                    nc.sync.dma_start(out=bet_r, in_=bet)
                    gam_b = cm.tile([128, D], F32)
                    bet_b = cm.tile([128, D], F32)
                    nc.gpsimd.partition_broadcast(gam_b, gam_r)
                    nc.gpsimd.partition_broadcast(bet_b, bet_r)

                for qt in range(NFT):
                    qts = slice(qt * 128, (qt + 1) * 128)
                    qres_t = cq.tile([128, D], F32, tag="qres")
                    nc.scalar.dma_start(out=qres_t, in_=qres[qts, :])
                    ps = cps.tile([128, D], F32, tag="x")
                    for c in range(2):
                        cs = slice(c * 512, (c + 1) * 512)
                        for ki in range(NFT):
                            nc.tensor.matmul(ps[:, cs], attn_sb[:, ki, qts],
                                             wo_m[:, ki, cs],
                                             start=(ki == 0),
                                             stop=(trivial_bias and ki == NFT - 1))
                        if not trivial_bias:
                            nc.tensor.matmul(ps[:, cs], ones[0:1, qts],
                                             wo_b[0:1, cs], start=False, stop=True)
                    x_sb = cq.tile([128, D], F32, tag="x_sb")
                    nc.vector.tensor_add(x_sb, ps, qres_t)

                    stats = cl.tile([128, 2, 6], F32, tag="stats")
                    nc.vector.bn_stats(stats[:, 0, :], x_sb[:, 0:512])
                    nc.vector.bn_stats(stats[:, 1, :], x_sb[:, 512:1024])
                    mv = cl.tile([128, 2], F32, tag="mv")
                    nc.vector.bn_aggr(mv, stats)
                    sq = cl.tile([128, 1], F32, tag="sq")
                    nc.scalar.activation(sq, mv[:, 1:2], AF.Sqrt, bias=eps_t)
                    rstd = cl.tile([128, 1], F32, tag="rstd")
                    nc.vector.reciprocal(rstd, sq)
                    xo = cl.tile([128, D], F32, tag="xo")
                    nc.vector.tensor_scalar(xo, x_sb, mv[:, 0:1], rstd,
                                            op0=mybir.AluOpType.subtract,
                                            op1=mybir.AluOpType.mult)
                    if not trivial_affine:
                        nc.vector.tensor_mul(xo, xo, gam_b)
                        nc.vector.tensor_add(xo, xo, bet_b)
                    nc.sync.dma_start(out=out[qts, :], in_=xo)
    return nc


def _prep_core_inputs(inputs, b, qh):
    """Build the per-core input map (host-side layout prep only)."""
    import ml_dtypes
    bf = ml_dtypes.bfloat16
    q = np.asarray(inputs["q"], np.float32)
    k = np.asarray(inputs["k"], np.float32)
    v = np.asarray(inputs["v"], np.float32)
    mask = np.asarray(inputs["attn_mask"])
    Wq, bq = np.asarray(inputs["Wq"], np.float32), np.asarray(inputs["bq"], np.float32)
    Wk, bk = np.asarray(inputs["Wk"], np.float32), np.asarray(inputs["bk"], np.float32)
    Wv, bv = np.asarray(inputs["Wv"], np.float32), np.asarray(inputs["bv"], np.float32)
    Wo, bo = np.asarray(inputs["Wo"], np.float32), np.asarray(inputs["bo"], np.float32)
    gamma, beta = np.asarray(inputs["gamma"], np.float32), np.asarray(inputs["beta"], np.float32)

    qs = slice(qh * TQ, (qh + 1) * TQ)
    qb = q[b, qs, :]                       # [TQ, D]

    def ext_T(x_t):  # [D, N] -> [D+1, N] with ones row
        return np.concatenate([x_t, np.ones((1, x_t.shape[1]), np.float32)], axis=0)

    def ext_W(W, bias):  # [D, N] -> [D+1, N] with bias row
        return np.concatenate([W, bias[None, :]], axis=0)

    # Wv extended with per-head ones column: col h*65+64 gets bias 1, weights 0
    Wv_ext = np.zeros((D, VEXT), np.float32)
    bv_ext = np.zeros((VEXT,), np.float32)
    for h in range(H):
        Wv_ext[:, h * 65:h * 65 + 64] = Wv[:, h * 64:(h + 1) * 64]
        bv_ext[h * 65:h * 65 + 64] = bv[h * 64:(h + 1) * 64]
        bv_ext[h * 65 + 64] = 1.0

    return {
        "qT": ext_T(qb.T.copy()).astype(bf),
        "kT": ext_T(k[b].T.copy()).astype(bf),
        "vT": ext_T(v[b].T.copy()).astype(bf),
        "wq": ext_W(Wq, bq).astype(bf),
        "wk": ext_W(Wk, bk).astype(bf),
        "wv": ext_W(Wv_ext, bv_ext).astype(bf),
        "wo": ext_W(Wo, bo).astype(bf),
        "maskT": np.ascontiguousarray(mask[b, qs, :].T).astype(bf),
        "qres": np.ascontiguousarray(qb),
        "gam": gamma[None, :].copy(),
        "bet": beta[None, :].copy(),
    }


def kernel(**inputs) -> np.ndarray:
    global _LAST_RESULTS
    trivial_affine = (np.all(np.asarray(inputs["gamma"]) == 1.0)
                      and np.all(np.asarray(inputs["beta"]) == 0.0))
    trivial_bias = all(
        np.all(np.asarray(inputs[k]) == 0.0) for k in ("bq", "bk", "bv", "bo"))
    nc = bacc.Bacc("TRN2", debug=False, num_devices=NCORES)
    build_program(nc, trivial_affine=trivial_affine, trivial_bias=trivial_bias)
    nc.finalize()

    ncores_run = int(os.environ.get("KERNEL_NCORES", str(NCORES)))
    in_maps = [_prep_core_inputs(inputs, c // 2, c % 2) for c in range(NCORES)]
    trace = bool(int(os.environ.get("KERNEL_TRACE", "0")))
    res = run_bass_kernel_spmd(nc, in_maps[:ncores_run],
                               core_ids=list(range(ncores_run)), trace=trace)
    _LAST_RESULTS = {"exec_time_ns": res.exec_time_ns,
                     "profile_json": res.profile_json,
                     "res": res}

    out = np.empty((B, T, D), np.float32)
    for c in range(NCORES):
        b, qh = c // 2, c % 2
        out[b, qh * TQ:(qh + 1) * TQ, :] = res.results[c % ncores_run]["out"]
    return out

